# revision 9
# baseline (speedup 1.0000x reference)
"""Neural-HMM forward kernel for Trainium2 (8 NeuronCores, SPMD data-parallel over batch).

Math: per (b, t) a 64x64 transition matrix A_t = 0.5*softmax(emb@W + b) +
0.5*softmax(unnorm_trans); the scan h_t = logsumexp_i(h_{t-1}[i] + log A_t[i,j])
is run in plain probability space (A_t is row-stochastic, mass conserved):
p_t = A_t^T p_{t-1}, h = log p (log taken on host).

Structure per core (2 batch rows, 4 blocks of 512 timesteps):
  stage 1  logits via fp8-e4m3 DoubleRow matmuls (K=1024 as 4x256), Exp on ACT
           into e[(h,i), (q, t)] with contiguous 128-channel writes, rowsum S
           accumulated incrementally on DVE, A_nn = E * (0.5/S).
  relayout SBUF->SBUF DMA permutes e[(h,i), (q,t)] -> av[(hh,i), (j, t')]
           (512B runs); + 0.5*trans added on DVE; strided LDWEIGHTS reads of
           av are free on the PE.
  stage 2  chunked prefix chains: 8 chunks x 64 steps per block,
           N_s = A^T N_{s-1} on the PE (two 64x64 quadrants via tile_position),
           PSUM->SBUF copies split across DVE (half) and ACT (half).
  stage 3  serial boundary scan over chunk products (tiny PE matmuls).
  stage 4  p = N q expansion on DVE; p (bf16) DMA'd out; host takes log.
Stage-1 matmuls of block b+1 are interleaved into the chain loop of block b
to keep the PE busy during the chain's PSUM round-trips.
"""

import numpy as np
import ml_dtypes
import sys

sys.path.insert(0, "/opt/trn_rl_repo")

import concourse.bass as bass
import concourse.bacc as bacc
import concourse.tile as tile
from concourse import mybir
from concourse.bass_utils import run_bass_kernel_spmd

F32 = mybir.dt.float32
BF16 = mybir.dt.bfloat16
FP8 = mybir.dt.float8e4

B, T, D, H = 16, 1024, 1024, 64
NCORES = 8
BLOC = B // NCORES          # 2 batch rows per core
NBLK = 4                    # blocks per core (bloc, t-half)
BT = 512                    # timesteps per block
NQ = 32                     # stage-1 m-tiles (128 HH-columns each)
NK2 = 4                     # fp8 DoubleRow contraction tiles (256 each)
L = 64                      # chain steps per chunk
NF = 4                      # chunks per partition half; total 8 chunks/block


def build_bass():
    nc = bacc.Bacc(
        "TRN2", target_bir_lowering=False, debug=False, num_devices=NCORES
    )
    embq = nc.declare_dram_parameter("embq", [128, NK2 * 2 * BLOC * T], FP8, isOutput=False)
    wq = nc.declare_dram_parameter("wq", [128, NK2 * 2 * H * H], FP8, isOutput=False)
    bq = nc.declare_dram_parameter("bq", [128, NQ], F32, isOutput=False)
    transq = nc.declare_dram_parameter("transq", [128, H], BF16, isOutput=False)
    identb = nc.declare_dram_parameter("identb", [128, H], BF16, isOutput=False)
    identf = nc.declare_dram_parameter("identf", [H, H], F32, isOutput=False)
    onesr = nc.declare_dram_parameter("onesr", [1, 128], F32, isOutput=False)
    priors_col = nc.declare_dram_parameter("priors_col", [128, 1], F32, isOutput=False)
    priors_row = nc.declare_dram_parameter("priors_row", [1, H], F32, isOutput=False)
    outp = nc.declare_dram_parameter("outp", [128, NBLK * 256], BF16, isOutput=True)

    from contextlib import ExitStack

    with tile.TileContext(nc) as tc, ExitStack() as ctx:
        kernel_body(ctx, tc, embq, wq, bq, transq, identb, identf, onesr,
                    priors_col, priors_row, outp)
    nc.finalize()
    return nc


def kernel_body(ctx, tc, embq, wq, bq, transq, identb, identf, onesr,
                priors_col, priors_row, outp):
    nc = tc.nc

    const_pool = ctx.enter_context(tc.tile_pool(name="const", bufs=1))
    big_pool = ctx.enter_context(tc.tile_pool(name="big", bufs=1))
    av_pool = ctx.enter_context(tc.tile_pool(name="av", bufs=2))
    s_pool = ctx.enter_context(tc.tile_pool(name="s", bufs=2))
    q_pool = ctx.enter_context(tc.tile_pool(name="q", bufs=4))
    qrep_pool = ctx.enter_context(tc.tile_pool(name="qrep", bufs=2))
    prod_pool = ctx.enter_context(tc.tile_pool(name="prod", bufs=1))
    mm_psum = ctx.enter_context(tc.tile_pool(name="mmps", bufs=2, space="PSUM"))
    ch_psum = ctx.enter_context(tc.tile_pool(name="chps", bufs=2, space="PSUM"))
    bnd_psum = ctx.enter_context(tc.tile_pool(name="bndps", bufs=2, space="PSUM"))

    # ---- constants
    transq_sb = const_pool.tile([128, H], BF16)
    nc.sync.dma_start(transq_sb[:, :], transq[:, :])
    ident_dma = const_pool.tile([128, H], BF16)
    nc.sync.dma_start(ident_dma[:, :], identb[:, :])
    ident_sb = const_pool.tile([128, H], BF16)
    nc.vector.tensor_copy(ident_sb[:, :], ident_dma[:, :])
    identf_dma = const_pool.tile([H, H], F32)
    nc.sync.dma_start(identf_dma[:, :], identf[:, :])
    identf_sb = const_pool.tile([H, H], F32)
    nc.vector.tensor_copy(identf_sb[:, :], identf_dma[:, :])
    ones_dma = const_pool.tile([1, 128], F32)
    nc.sync.dma_start(ones_dma[:, :], onesr[:, :])
    ones_sb = const_pool.tile([1, 128], F32)
    nc.vector.tensor_copy(ones_sb[:, :], ones_dma[:, :])
    pcol_sb = const_pool.tile([128, 1], F32)
    nc.sync.dma_start(pcol_sb[:, :], priors_col[:, :])
    prow_sb = const_pool.tile([1, H], F32)
    nc.sync.dma_start(prow_sb[:, :], priors_row[:, :])
    bq_sb = const_pool.tile([128, NQ], F32)
    nc.sync.dma_start(bq_sb[:, :], bq[:, :])

    # trans pre-expanded over a half t'-range: trx[p, j, t128] = 0.5*trans[i, j]
    trx = const_pool.tile([128, H * 64], BF16)
    nc.vector.tensor_copy(
        trx[:, :].rearrange("p (j t) -> p j t", j=H),
        transq_sb[:, :].rearrange("p (j u) -> p j u", u=1).broadcast_to([128, H, 64]),
    )

    # ---- resident inputs
    wq_sb = big_pool.tile([128, NK2 * 2 * H * H], FP8)
    nc.sync.dma_start(wq_sb[:, :], wq[:, :])
    embq_sb = big_pool.tile([128, NK2 * 2 * BLOC * T], FP8)
    nc.sync.dma_start(embq_sb[:, :], embq[:, :])
    wqv = wq_sb[:, :].rearrange("p (k u m) -> p k u m", k=NK2, u=2)
    embqv = embq_sb[:, :].rearrange("p (k u t) -> p k u t", k=NK2, u=2)

    e_sb = big_pool.tile([128, NQ * BT], BF16)      # [(h,i), (q, t)]
    ev = e_sb[:, :].rearrange("p (q t) -> p q t", q=NQ)
    nv = big_pool.tile([128, L * NF * H], BF16)     # [(hh,j), (s, f, c)]
    nvv = nv[:, :].rearrange("p (s f c) -> p s f c", s=L, f=NF)
    out_sb = big_pool.tile([128, NBLK * 256], BF16)

    # ---- per-block state helpers
    state = {}

    def stage1_group(blk, q):
        ps = mm_psum.tile([128, BT], F32, tag="mm")
        for k2 in range(NK2):
            nc.tensor.matmul(
                ps[:, :],
                wqv[:, k2, :, q * 128 : (q + 1) * 128],
                embqv[:, k2, :, blk * BT : (blk + 1) * BT],
                start=(k2 == 0),
                stop=(k2 == NK2 - 1),
                perf_mode=mybir.MatmulPerfMode.DoubleRow,
            )
        nc.scalar.activation(
            ev[:, q, :], ps[:, :], mybir.ActivationFunctionType.Exp,
            bias=bq_sb[:, q : q + 1],
        )

    def make_av(blk):
        av = av_pool.tile([128, H * 256], BF16, tag="av")
        return av

    def relayout(blk, av):
        """SBUF->SBUF DMA: e[(h,i),(q,t)] -> av[(hh,i),(j=q+32h, t')] (raw exps)."""
        dv = av[:, :].rearrange("p (j t) -> p j t", j=H)
        for hh in range(2):
            for h in range(2):
                eng = nc.sync if h == 0 else nc.gpsimd
                eng.dma_start(
                    dv[hh * 64 : hh * 64 + 64, 32 * h : 32 * h + 32, :],
                    ev[h * 64 : h * 64 + 64, :, hh * 256 : hh * 256 + 256],
                )

    def mix_on_av(blk, av):
        """rowsum over j (tree, e as scratch), r = 0.5/S, av = av*r + 0.5*trans."""
        dv = av[:, :].rearrange("p (j t) -> p j t", j=H)
        ctx2 = nc.allow_low_precision(reason="softmax denom tree in bf16")
        ctx2.__enter__()
        # 6-level tree: S[(hh,i), t'] = sum_j av[(hh,i), j, t']
        nc.vector.tensor_tensor(
            e_sb[:, 0:8192].rearrange("p (j t) -> p j t", j=32),
            dv[:, 0:32, :], dv[:, 32:64, :], op=mybir.AluOpType.add,
        )
        for w in (4096, 2048, 1024, 512):
            nc.vector.tensor_tensor(
                e_sb[:, 0:w], e_sb[:, 0:w], e_sb[:, w : 2 * w], op=mybir.AluOpType.add,
            )
        s32 = s_pool.tile([128, 256], F32, tag="s32")
        nc.vector.tensor_tensor(s32[:, :], e_sb[:, 0:256], e_sb[:, 256:512], op=mybir.AluOpType.add)
        nc.vector.reciprocal(s32[:, :], s32[:, :])
        r_sb = s_pool.tile([128, 256], BF16, tag="r")
        nc.vector.tensor_scalar_mul(r_sb[:, :], s32[:, :], 0.5)
        nc.vector.tensor_tensor(
            dv[:, :, :], dv[:, :, :],
            r_sb[:, :].rearrange("p (u t) -> p u t", u=1).broadcast_to([128, H, 256]),
            op=mybir.AluOpType.mult,
        )
        trxv = trx[:, :].rearrange("p (j t) -> p j t", j=H)
        for quarter in range(4):
            nc.vector.tensor_tensor(
                dv[:, :, quarter * 64 : quarter * 64 + 64],
                dv[:, :, quarter * 64 : quarter * 64 + 64],
                trxv[:, :, :],
                op=mybir.AluOpType.add,
            )
        ctx2.__exit__(None, None, None)

    def reset_q():
        q_col = q_pool.tile([128, 1], F32, tag="qcol")
        nc.vector.tensor_copy(q_col[:, :], pcol_sb[:, :])
        q_row = q_pool.tile([1, H], F32, tag="qrow")
        nc.vector.tensor_copy(q_row[:, :], prow_sb[:, :])
        state["q_col"] = q_col
        state["q_row"] = q_row

    def chains(blk, av, fillers, after_fillers=None, mix_fn=None):
        th = blk % 2
        avv = av[:, :].rearrange("p (j t) -> p j t", j=H)
        first_global = th == 0
        mix_pending = mix_fn is not None
        for s in range(L):
            cp = ch_psum.tile([128, 256], F32, tag="chp")
            cpv = cp[:, :].rearrange("p (f c) -> p f c", f=NF)
            for f in range(NF):
                for hh in range(2):
                    if s == 0:
                        if first_global and hh == 0 and f == 0:
                            continue
                        rhs = ident_sb[hh * 64 : hh * 64 + 64, :]
                    else:
                        rhs = nvv[hh * 64 : hh * 64 + 64, s - 1, f, :]
                    nc.tensor.matmul(
                        cp[hh * 64 : hh * 64 + 64, f * 64 : f * 64 + 64],
                        avv[hh * 64 : hh * 64 + 64, :, f * 64 + s],
                        rhs,
                        start=True,
                        stop=True,
                        tile_position=(hh * 64, hh * 64),
                    )
            # copy N_s psum->nv: DVE is busy with expansion (slot start) or
            # the next block's mix (mid-slot) in some windows -> ACT-only there
            act_both = (s < 12) or (mix_pending and 24 <= s < 52)
            if s == 0 and first_global:
                nc.vector.tensor_copy(nvv[0:64, 0, 0, :], ident_sb[0:64, :])
                nc.scalar.activation(
                    nvv[0:64, 0, 1:4, :], cpv[0:64, 1:4, :],
                    mybir.ActivationFunctionType.Copy,
                )
                nc.scalar.activation(
                    nvv[64:128, 0, :, :], cpv[64:128, :, :],
                    mybir.ActivationFunctionType.Copy,
                )
            elif act_both:
                nc.scalar.activation(
                    nvv[0:64, s, :, :], cpv[0:64, :, :],
                    mybir.ActivationFunctionType.Copy,
                )
                nc.scalar.activation(
                    nvv[64:128, s, :, :], cpv[64:128, :, :],
                    mybir.ActivationFunctionType.Copy,
                )
            else:
                nc.vector.tensor_copy(nvv[0:64, s, :, :], cpv[0:64, :, :])
                nc.scalar.activation(
                    nvv[64:128, s, :, :], cpv[64:128, :, :],
                    mybir.ActivationFunctionType.Copy,
                )
            if fillers:
                fillers.pop(0)()
                if not fillers and after_fillers is not None:
                    after_fillers()
                    after_fillers = None
            elif mix_pending and s >= 24 and mix_fn is not None:
                mix_fn()
                mix_fn = None
        if after_fillers is not None:
            after_fillers()
        if mix_fn is not None:
            mix_fn()

    def boundary_expansion(blk, av):
        avv = av[:, :].rearrange("p (j t) -> p j t", j=H)
        qreps = []
        for f in range(NF):
            qr = qrep_pool.tile([128, H], BF16, tag=f"qr{f}")
            qreps.append(qr)
        state["qreps"] = qreps
        q_col = state["q_col"]
        q_row = state["q_row"]
        for c in range(2 * NF):
            hh, f = c // NF, c % NF
            bnd = bnd_psum.tile([128, 256], F32, tag="bnd")
            nc.tensor.matmul(bnd[:, 0:H], ones_sb[:, :], q_row[:, :], start=True, stop=True)
            nc.scalar.activation(
                qreps[f][hh * 64 : hh * 64 + 64, :], bnd[hh * 64 : hh * 64 + 64, 0:H],
                mybir.ActivationFunctionType.Copy,
            )
            # full chunk product M63 = N_62^T-chain x A_63
            nc.tensor.matmul(
                bnd[hh * 64 : hh * 64 + 64, H : 2 * H],
                nvv[hh * 64 : hh * 64 + 64, L - 2, f, :],
                avv[hh * 64 : hh * 64 + 64, :, f * 64 + (L - 1)],
                start=True,
                stop=True,
                tile_position=(hh * 64, hh * 64),
            )
            m63_sb = q_pool.tile([128, H], F32, tag="m63")
            nc.vector.tensor_copy(
                m63_sb[hh * 64 : hh * 64 + 64, :], bnd[hh * 64 : hh * 64 + 64, H : 2 * H]
            )
            nc.tensor.matmul(
                bnd[0:H, 128:129],
                m63_sb[hh * 64 : hh * 64 + 64, :],
                q_col[hh * 64 : hh * 64 + 64, :],
                start=True,
                stop=True,
                tile_position=(hh * 64, 0),
            )
            q_col = q_pool.tile([128, 1], F32, tag="qcol")
            nc.vector.tensor_copy(q_col[0:64, :], bnd[0:H, 128:129])
            nc.vector.tensor_copy(q_col[64:128, :], bnd[0:H, 128:129])
            nc.tensor.matmul(
                bnd[0:1, 192:256], q_col[0:64, :], identf_sb[:, :], start=True, stop=True
            )
            q_row = q_pool.tile([1, H], F32, tag="qrow")
            nc.vector.tensor_copy(q_row[:, :], bnd[0:1, 192:256])
            if c >= NF:
                expansion_f(blk, c - NF)
        state["q_col"] = q_col
        state["q_row"] = q_row

    def expansion_f(blk, f):
        qreps = state["qreps"]
        with nc.allow_low_precision(reason="p in bf16; log on host"):
            for sh2 in range(2):
                prod = prod_pool.tile([128, (L // 2) * H], BF16, tag="prod")
                pv = prod[:, :].rearrange("p (s c) -> p s c", s=L // 2)
                nc.vector.tensor_tensor(
                    pv,
                    nvv[:, sh2 * 32 : sh2 * 32 + 32, f, :],
                    qreps[f][:, :].rearrange("p (u c) -> p u c", u=1).broadcast_to([128, L // 2, H]),
                    op=mybir.AluOpType.mult,
                )
                o0 = blk * 256 + f * 64 + sh2 * 32
                nc.vector.tensor_reduce(
                    out=out_sb[:, o0 : o0 + 32],
                    in_=pv,
                    axis=mybir.AxisListType.X,
                    op=mybir.AluOpType.add,
                )

    # ---- pipelined schedule
    # prologue: stage 1 of block 0
    av_cur = make_av(0)
    for q in range(NQ):
        stage1_group(0, q)
    relayout(0, av_cur)
    mix_on_av(0, av_cur)
    reset_q()

    for b in range(NBLK):
        if b == 2:
            reset_q()
        if b + 1 < NBLK:
            av_next = make_av(b + 1)
            fillers = [
                (lambda q=q, nb=b + 1: stage1_group(nb, q)) for q in range(NQ)
            ]
            after = lambda nb=b + 1, avn=av_next: relayout(nb, avn)
        else:
            av_next = None
            fillers = []
            after = None
        # pre-drain some of the next block's stage 1 to cover pipeline stalls
        for _ in range(12):
            if fillers:
                fillers.pop(0)()
        if b + 1 < NBLK:
            mix_fn = lambda nb=b + 1, avn=av_next: mix_on_av(nb, avn)
        else:
            mix_fn = None
        chains(b, av_cur, fillers, after, mix_fn)
        boundary_expansion(b, av_cur)
        if b + 1 < NBLK:
            av_cur = av_next

    nc.sync.dma_start(outp[:, :], out_sb[:, :])


def kernel(emb, W, b, unnorm_trans, state_priors):
    emb = np.asarray(emb, dtype=np.float32)
    W = np.asarray(W, dtype=np.float32)
    b = np.asarray(b, dtype=np.float32)
    unnorm_trans = np.asarray(unnorm_trans, dtype=np.float32)
    state_priors = np.asarray(state_priors, dtype=np.float32)

    E4 = ml_dtypes.float8_e4m3

    def to_e4(x):
        return np.clip(x, -240.0, 240.0).astype(E4)

    # host-side constants
    ut = unnorm_trans - unnorm_trans.max(axis=-1, keepdims=True)
    e = np.exp(ut)
    trans_half = (0.5 * e / e.sum(axis=-1, keepdims=True)).astype(np.float32)
    transq = np.tile(trans_half, (2, 1)).astype(ml_dtypes.bfloat16)
    identb = np.tile(np.eye(H, dtype=np.float32), (2, 1)).astype(ml_dtypes.bfloat16)
    identf = np.eye(H, dtype=np.float32)
    onesr = np.ones((1, 128), dtype=np.float32)
    pr = np.exp(state_priors).astype(np.float32)
    priors_col = np.tile(pr[:, None], (2, 1)).astype(np.float32)
    priors_row = pr[None, :].astype(np.float32)

    # W -> wq[p, k2, u, q*128 + h*64 + i] = W[k2*256+u*128+p, i*64 + q + 32h]
    Wr = W.reshape(D, H, 2, 32)                # [d, i, h, q]
    Wp = Wr.transpose(0, 3, 2, 1).reshape(D, H * H)  # [d, (q, h, i)]
    wq = np.ascontiguousarray(
        Wp.reshape(NK2, 2, 128, H * H).transpose(2, 0, 1, 3).reshape(128, NK2 * 2 * H * H)
    )
    wq = to_e4(wq)
    # bias bq[h*64+i, q] = b[i*64 + q + 32h]
    br = b.reshape(H, 2, 32)                   # [i, h, q]
    bq = np.ascontiguousarray(br.transpose(1, 0, 2).reshape(128, 32)).astype(np.float32)

    nc = build_bass()

    in_maps = []
    for core in range(NCORES):
        emb_c = emb[core * BLOC : (core + 1) * BLOC]       # [2, T, D]
        ef = emb_c.reshape(BLOC * T, D).T                  # [d, tg]
        embq = np.ascontiguousarray(
            ef.reshape(NK2, 2, 128, BLOC * T).transpose(2, 0, 1, 3).reshape(128, -1)
        )
        in_maps.append(
            {
                "embq": to_e4(embq),
                "wq": wq,
                "bq": bq,
                "transq": transq,
                "identb": identb,
                "identf": identf,
                "onesr": onesr,
                "priors_col": priors_col,
                "priors_row": priors_row,
            }
        )

    import os

    trace = bool(int(os.environ.get("KERNEL_TRACE", "0")))
    res = run_bass_kernel_spmd(nc, in_maps, list(range(NCORES)), trace=trace)
    global LAST_RESULTS
    LAST_RESULTS = res
    if trace and res.exec_time_ns is not None:
        print(f"HW exec time: {res.exec_time_ns} ns")
        print(f"  mean across cores: {res.mean_exec_time_ns} ns")

    outs = []
    for core in range(NCORES):
        o = res.results[core]["outp"].astype(np.float32)   # [128, 1024]
        o = o.reshape(2, 64, NBLK, NF, L)                  # [hh, j, blk, f, s]
        arr = o.transpose(2, 0, 3, 4, 1)                   # [blk, hh, f, s, j]
        arr = arr.reshape(BLOC, T, H)                      # blk=(bloc,th); t=(th,hh,f,s)
        outs.append(arr)
    h = np.log(np.maximum(np.concatenate(outs, axis=0), 1e-30)).astype(np.float32)
    h[:, 0, :] = state_priors[None, :]
    return h


# revision 12
# speedup vs baseline: 1.1105x; 1.1105x over previous
"""Neural-HMM forward kernel for Trainium2 (8 NeuronCores, SPMD data-parallel over batch).

Math: per (b, t) a 64x64 transition matrix A_t = 0.5*softmax(emb@W + b) +
0.5*softmax(unnorm_trans); the scan h_t = logsumexp_i(h_{t-1}[i] + log A_t[i,j])
is run in plain probability space (A_t is row-stochastic, mass conserved):
p_t = A_t^T p_{t-1}, h = log p (log taken on host).

Structure per core (2 batch rows, 4 blocks of 512 timesteps):
  stage 1  logits via fp8-e4m3 DoubleRow matmuls (K=1024 as 4x256), Exp on ACT
           into e[(h,i), (q, t)] with contiguous 128-channel writes, rowsum S
           accumulated incrementally on DVE, A_nn = E * (0.5/S).
  relayout SBUF->SBUF DMA permutes e[(h,i), (q,t)] -> av[(hh,i), (j, t')]
           (512B runs); + 0.5*trans added on DVE; strided LDWEIGHTS reads of
           av are free on the PE.
  stage 2  chunked prefix chains: 8 chunks x 64 steps per block,
           N_s = A^T N_{s-1} on the PE (two 64x64 quadrants via tile_position),
           PSUM->SBUF copies split across DVE (half) and ACT (half).
  stage 3  serial boundary scan over chunk products (tiny PE matmuls).
  stage 4  p = N q expansion on DVE; p (bf16) DMA'd out; host takes log.
Stage-1 matmuls of block b+1 are interleaved into the chain loop of block b
to keep the PE busy during the chain's PSUM round-trips.
"""

import numpy as np
import ml_dtypes
import sys

sys.path.insert(0, "/opt/trn_rl_repo")

import concourse.bass as bass
import concourse.bacc as bacc
import concourse.tile as tile
from concourse import mybir
from concourse.bass_utils import run_bass_kernel_spmd

F32 = mybir.dt.float32
BF16 = mybir.dt.bfloat16
FP8 = mybir.dt.float8e4

B, T, D, H = 16, 1024, 1024, 64
NCORES = 8
BLOC = B // NCORES          # 2 batch rows per core
NBLK = 4                    # blocks per core (bloc, t-half)
BT = 512                    # timesteps per block
NQ = 32                     # stage-1 m-tiles (128 HH-columns each)
NK2 = 4                     # fp8 DoubleRow contraction tiles (256 each)
L = 64                      # chain steps per chunk
NF = 4                      # chunks per partition half; total 8 chunks/block


def build_bass():
    nc = bacc.Bacc(
        "TRN2", target_bir_lowering=False, debug=False, num_devices=NCORES
    )
    embq = nc.declare_dram_parameter("embq", [128, NK2 * 2 * BLOC * T], FP8, isOutput=False)
    wq = nc.declare_dram_parameter("wq", [128, NK2 * 2 * H * H], FP8, isOutput=False)
    bq = nc.declare_dram_parameter("bq", [128, NQ], F32, isOutput=False)
    transq = nc.declare_dram_parameter("transq", [128, H], BF16, isOutput=False)
    identb = nc.declare_dram_parameter("identb", [128, H], BF16, isOutput=False)
    identf = nc.declare_dram_parameter("identf", [H, H], F32, isOutput=False)
    onesr = nc.declare_dram_parameter("onesr", [1, 128], F32, isOutput=False)
    priors_col = nc.declare_dram_parameter("priors_col", [128, 1], F32, isOutput=False)
    priors_row = nc.declare_dram_parameter("priors_row", [1, H], F32, isOutput=False)
    outp = nc.declare_dram_parameter("outp", [128, NBLK * 256], BF16, isOutput=True)

    from contextlib import ExitStack

    with tile.TileContext(nc) as tc, ExitStack() as ctx:
        kernel_body(ctx, tc, embq, wq, bq, transq, identb, identf, onesr,
                    priors_col, priors_row, outp)
    nc.finalize()
    return nc


def kernel_body(ctx, tc, embq, wq, bq, transq, identb, identf, onesr,
                priors_col, priors_row, outp):
    nc = tc.nc

    const_pool = ctx.enter_context(tc.tile_pool(name="const", bufs=1))
    big_pool = ctx.enter_context(tc.tile_pool(name="big", bufs=1))
    av_pool = ctx.enter_context(tc.tile_pool(name="av", bufs=2))
    s_pool = ctx.enter_context(tc.tile_pool(name="s", bufs=2))
    q_pool = ctx.enter_context(tc.tile_pool(name="q", bufs=4))
    qrep_pool = ctx.enter_context(tc.tile_pool(name="qrep", bufs=2))
    prod_pool = ctx.enter_context(tc.tile_pool(name="prod", bufs=1))
    mm_psum = ctx.enter_context(tc.tile_pool(name="mmps", bufs=2, space="PSUM"))
    ch_psum = ctx.enter_context(tc.tile_pool(name="chps", bufs=2, space="PSUM"))
    bnd_psum = ctx.enter_context(tc.tile_pool(name="bndps", bufs=2, space="PSUM"))

    # ---- constants
    transq_sb = const_pool.tile([128, H], BF16)
    nc.sync.dma_start(transq_sb[:, :], transq[:, :])
    ident_dma = const_pool.tile([128, H], BF16)
    nc.sync.dma_start(ident_dma[:, :], identb[:, :])
    ident_sb = const_pool.tile([128, H], BF16)
    nc.vector.tensor_copy(ident_sb[:, :], ident_dma[:, :])
    identf_dma = const_pool.tile([H, H], F32)
    nc.sync.dma_start(identf_dma[:, :], identf[:, :])
    identf_sb = const_pool.tile([H, H], F32)
    nc.vector.tensor_copy(identf_sb[:, :], identf_dma[:, :])
    ones_dma = const_pool.tile([1, 128], F32)
    nc.sync.dma_start(ones_dma[:, :], onesr[:, :])
    ones_sb = const_pool.tile([1, 128], F32)
    nc.vector.tensor_copy(ones_sb[:, :], ones_dma[:, :])
    pcol_sb = const_pool.tile([128, 1], F32)
    nc.sync.dma_start(pcol_sb[:, :], priors_col[:, :])
    prow_sb = const_pool.tile([1, H], F32)
    nc.sync.dma_start(prow_sb[:, :], priors_row[:, :])
    bq_sb = const_pool.tile([128, NQ], F32)
    nc.sync.dma_start(bq_sb[:, :], bq[:, :])

    # trans pre-expanded over a half t'-range: trx[p, j, t128] = 0.5*trans[i, j]
    trx = const_pool.tile([128, H * 64], BF16)
    nc.vector.tensor_copy(
        trx[:, :].rearrange("p (j t) -> p j t", j=H),
        transq_sb[:, :].rearrange("p (j u) -> p j u", u=1).broadcast_to([128, H, 64]),
    )

    # ---- resident inputs
    wq_sb = big_pool.tile([128, NK2 * 2 * H * H], FP8)
    nc.sync.dma_start(wq_sb[:, :], wq[:, :])
    embq_sb = big_pool.tile([128, NK2 * 2 * BLOC * T], FP8)
    nc.sync.dma_start(embq_sb[:, :], embq[:, :])
    wqv = wq_sb[:, :].rearrange("p (k u m) -> p k u m", k=NK2, u=2)
    embqv = embq_sb[:, :].rearrange("p (k u t) -> p k u t", k=NK2, u=2)

    e_sb = big_pool.tile([128, NQ * BT], BF16)      # [(h,i), (q, t)]
    ev = e_sb[:, :].rearrange("p (q t) -> p q t", q=NQ)
    nv = big_pool.tile([128, L * NF * H], BF16)     # [(hh,j), (s, f, c)]
    nvv = nv[:, :].rearrange("p (s f c) -> p s f c", s=L, f=NF)
    HEAD = 12
    nv_head = big_pool.tile([128, HEAD * NF * H], BF16)
    nhv = nv_head[:, :].rearrange("p (s f c) -> p s f c", s=HEAD, f=NF)
    out_sb = big_pool.tile([128, NBLK * 256], BF16)

    # ---- per-block state helpers
    state = {}

    def _s1_mm(ps, blk, q, k2):
        nc.tensor.matmul(
            ps[:, :],
            wqv[:, k2, :, q * 128 : (q + 1) * 128],
            embqv[:, k2, :, blk * BT : (blk + 1) * BT],
            start=(k2 == 0),
            stop=(k2 == NK2 - 1),
            perf_mode=mybir.MatmulPerfMode.DoubleRow,
            skip_group_check=True,
        )

    def stage1_units(blk):
        units = []
        for q in range(NQ):
            box = {}
            def unit_a(q=q, box=box):
                ps = mm_psum.tile([128, BT], F32, tag="mm")
                box["ps"] = ps
                _s1_mm(ps, blk, q, 0)
                _s1_mm(ps, blk, q, 1)
            def unit_b(q=q, box=box):
                ps = box["ps"]
                _s1_mm(ps, blk, q, 2)
                _s1_mm(ps, blk, q, 3)
                nc.scalar.activation(
                    ev[:, q, :], ps[:, :], mybir.ActivationFunctionType.Exp,
                    bias=bq_sb[:, q : q + 1],
                )
            units.append(unit_a)
            units.append(unit_b)
        return units

    def stage1_group(blk, q):
        ps = mm_psum.tile([128, BT], F32, tag="mm")
        for k2 in range(NK2):
            _s1_mm(ps, blk, q, k2)
        nc.scalar.activation(
            ev[:, q, :], ps[:, :], mybir.ActivationFunctionType.Exp,
            bias=bq_sb[:, q : q + 1],
        )

    def make_av(blk):
        av = av_pool.tile([128, H * 256], BF16, tag="av")
        return av

    def relayout(blk, av):
        """SBUF->SBUF DMA: e[(h,i),(q,t)] -> av[(hh,i),(j=q+32h, t')] (raw exps)."""
        dv = av[:, :].rearrange("p (j t) -> p j t", j=H)
        for hh in range(2):
            for h in range(2):
                eng = nc.sync if h == 0 else nc.gpsimd
                eng.dma_start(
                    dv[hh * 64 : hh * 64 + 64, 32 * h : 32 * h + 32, :],
                    ev[h * 64 : h * 64 + 64, :, hh * 256 : hh * 256 + 256],
                )

    def mix_on_av(blk, av):
        """rowsum over j (tree, e as scratch), r = 0.5/S, av = av*r + 0.5*trans."""
        dv = av[:, :].rearrange("p (j t) -> p j t", j=H)
        ctx2 = nc.allow_low_precision(reason="softmax denom tree in bf16")
        ctx2.__enter__()
        # 6-level tree: S[(hh,i), t'] = sum_j av[(hh,i), j, t']
        nc.vector.tensor_tensor(
            e_sb[:, 0:8192].rearrange("p (j t) -> p j t", j=32),
            dv[:, 0:32, :], dv[:, 32:64, :], op=mybir.AluOpType.add,
        )
        for w in (4096, 2048, 1024, 512):
            nc.vector.tensor_tensor(
                e_sb[:, 0:w], e_sb[:, 0:w], e_sb[:, w : 2 * w], op=mybir.AluOpType.add,
            )
        s32 = s_pool.tile([128, 256], F32, tag="s32")
        nc.vector.tensor_tensor(s32[:, :], e_sb[:, 0:256], e_sb[:, 256:512], op=mybir.AluOpType.add)
        nc.vector.reciprocal(s32[:, :], s32[:, :])
        r_sb = s_pool.tile([128, 256], BF16, tag="r")
        nc.vector.tensor_scalar_mul(r_sb[:, :], s32[:, :], 0.5)
        nc.vector.tensor_tensor(
            dv[:, :, :], dv[:, :, :],
            r_sb[:, :].rearrange("p (u t) -> p u t", u=1).broadcast_to([128, H, 256]),
            op=mybir.AluOpType.mult,
        )
        trxv = trx[:, :].rearrange("p (j t) -> p j t", j=H)
        for quarter in range(4):
            nc.vector.tensor_tensor(
                dv[:, :, quarter * 64 : quarter * 64 + 64],
                dv[:, :, quarter * 64 : quarter * 64 + 64],
                trxv[:, :, :],
                op=mybir.AluOpType.add,
            )
        ctx2.__exit__(None, None, None)

    def reset_q():
        q_col = q_pool.tile([128, 1], F32, tag="qcol")
        nc.vector.tensor_copy(q_col[:, :], pcol_sb[:, :])
        q_row = q_pool.tile([1, H], F32, tag="qrow")
        nc.vector.tensor_copy(q_row[:, :], prow_sb[:, :])
        state["q_col"] = q_col
        state["q_row"] = q_row

    def chains(blk, av, fillers, after_fillers=None, mix_fn=None, late_exp=None):
        th = blk % 2
        avv = av[:, :].rearrange("p (j t) -> p j t", j=H)
        first_global = th == 0
        has_mix = mix_fn is not None
        for s in range(L):
            cp = ch_psum.tile([128, 256], F32, tag="chp")
            cpv = cp[:, :].rearrange("p (f c) -> p f c", f=NF)
            for f in range(NF):
                for hh in range(2):
                    if s == 0:
                        if first_global and hh == 0 and f == 0:
                            continue
                        rhs = ident_sb[hh * 64 : hh * 64 + 64, :]
                    elif s <= HEAD:
                        rhs = nhv[hh * 64 : hh * 64 + 64, s - 1, f, :]
                    else:
                        rhs = nvv[hh * 64 : hh * 64 + 64, s - 1, f, :]
                    nc.tensor.matmul(
                        cp[hh * 64 : hh * 64 + 64, f * 64 : f * 64 + 64],
                        avv[hh * 64 : hh * 64 + 64, :, f * 64 + s],
                        rhs,
                        start=True,
                        stop=True,
                        tile_position=(hh * 64, hh * 64),
                    )
            # s < HEAD slots land in nv_head so the previous block's expansion
            # can still be reading nv; DVE-busy windows push copies to ACT.
            dstv = nhv if s < HEAD else nvv
            act_both = (s < HEAD) or (20 <= s < 36) or (has_mix and 40 <= s < 58)
            if s == 0 and first_global:
                nc.vector.tensor_copy(dstv[0:64, 0, 0, :], ident_sb[0:64, :])
                nc.scalar.activation(
                    dstv[0:64, 0, 1:4, :], cpv[0:64, 1:4, :],
                    mybir.ActivationFunctionType.Copy,
                )
                nc.scalar.activation(
                    dstv[64:128, 0, :, :], cpv[64:128, :, :],
                    mybir.ActivationFunctionType.Copy,
                )
            elif act_both:
                nc.scalar.activation(
                    dstv[0:64, s, :, :], cpv[0:64, :, :],
                    mybir.ActivationFunctionType.Copy,
                )
                nc.scalar.activation(
                    dstv[64:128, s, :, :], cpv[64:128, :, :],
                    mybir.ActivationFunctionType.Copy,
                )
            else:
                nc.vector.tensor_copy(dstv[0:64, s, :, :], cpv[0:64, :, :])
                nc.scalar.activation(
                    dstv[64:128, s, :, :], cpv[64:128, :, :],
                    mybir.ActivationFunctionType.Copy,
                )
            if s == HEAD:
                # drain staged head slots into nv (after old expansion done)
                nc.vector.tensor_copy(nv[:, 0 : HEAD * NF * H], nv_head[:, :])
            if s == 20 and late_exp is not None:
                late_exp()
                late_exp = None
            npop = 2 if s < 24 else 1
            for _ in range(npop):
                if fillers:
                    fillers.pop(0)()
                    if not fillers and after_fillers is not None:
                        after_fillers()
                        after_fillers = None
            if not fillers and mix_fn is not None and s >= 42:
                mix_fn()
                mix_fn = None
        if after_fillers is not None:
            after_fillers()
        if mix_fn is not None:
            mix_fn()
        if late_exp is not None:
            late_exp()

    def boundary_expansion(blk, av):
        avv = av[:, :].rearrange("p (j t) -> p j t", j=H)
        qreps = []
        for f in range(NF):
            qr = qrep_pool.tile([128, H], BF16, tag=f"qr{f}")
            qreps.append(qr)
        state["qreps"] = qreps
        q_col = state["q_col"]
        q_row = state["q_row"]
        for c in range(2 * NF):
            hh, f = c // NF, c % NF
            bnd = bnd_psum.tile([128, 256], F32, tag="bnd")
            nc.tensor.matmul(bnd[:, 0:H], ones_sb[:, :], q_row[:, :], start=True, stop=True)
            nc.scalar.activation(
                qreps[f][hh * 64 : hh * 64 + 64, :], bnd[hh * 64 : hh * 64 + 64, 0:H],
                mybir.ActivationFunctionType.Copy,
            )
            # full chunk product M63 = N_62^T-chain x A_63
            nc.tensor.matmul(
                bnd[hh * 64 : hh * 64 + 64, H : 2 * H],
                nvv[hh * 64 : hh * 64 + 64, L - 2, f, :],
                avv[hh * 64 : hh * 64 + 64, :, f * 64 + (L - 1)],
                start=True,
                stop=True,
                tile_position=(hh * 64, hh * 64),
            )
            m63_sb = q_pool.tile([128, H], F32, tag="m63")
            nc.vector.tensor_copy(
                m63_sb[hh * 64 : hh * 64 + 64, :], bnd[hh * 64 : hh * 64 + 64, H : 2 * H]
            )
            nc.tensor.matmul(
                bnd[0:H, 128:129],
                m63_sb[hh * 64 : hh * 64 + 64, :],
                q_col[hh * 64 : hh * 64 + 64, :],
                start=True,
                stop=True,
                tile_position=(hh * 64, 0),
            )
            q_col = q_pool.tile([128, 1], F32, tag="qcol")
            nc.vector.tensor_copy(q_col[0:64, :], bnd[0:H, 128:129])
            nc.vector.tensor_copy(q_col[64:128, :], bnd[0:H, 128:129])
            nc.tensor.matmul(
                bnd[0:1, 192:256], q_col[0:64, :], identf_sb[:, :], start=True, stop=True
            )
            q_row = q_pool.tile([1, H], F32, tag="qrow")
            nc.vector.tensor_copy(q_row[:, :], bnd[0:1, 192:256])
        state["q_col"] = q_col
        state["q_row"] = q_row

    def expansion_half(blk, sh2, qreps):
        with nc.allow_low_precision(reason="p in bf16; log on host"):
            for f in range(NF):
                prod = prod_pool.tile([128, (L // 2) * H], BF16, tag="prod")
                pv = prod[:, :].rearrange("p (s c) -> p s c", s=L // 2)
                nc.vector.tensor_tensor(
                    pv,
                    nvv[:, sh2 * 32 : sh2 * 32 + 32, f, :],
                    qreps[f][:, :].rearrange("p (u c) -> p u c", u=1).broadcast_to([128, L // 2, H]),
                    op=mybir.AluOpType.mult,
                )
                o0 = blk * 256 + f * 64 + sh2 * 32
                nc.vector.tensor_reduce(
                    out=out_sb[:, o0 : o0 + 32],
                    in_=pv,
                    axis=mybir.AxisListType.X,
                    op=mybir.AluOpType.add,
                )

    # ---- pipelined schedule
    # prologue: stage 1 of block 0
    av_cur = make_av(0)
    for q in range(NQ):
        stage1_group(0, q)
    relayout(0, av_cur)
    mix_on_av(0, av_cur)
    reset_q()

    late_exp = None
    for b in range(NBLK):
        if b == 2:
            reset_q()
        if b + 1 < NBLK:
            av_next = make_av(b + 1)
            fillers = stage1_units(b + 1)
            after = lambda nb=b + 1, avn=av_next: relayout(nb, avn)
            mix_fn = lambda nb=b + 1, avn=av_next: mix_on_av(nb, avn)
        else:
            av_next = None
            fillers = []
            after = None
            mix_fn = None
        chains(b, av_cur, fillers, after, mix_fn, late_exp)
        boundary_expansion(b, av_cur)
        qreps_b = state["qreps"]
        expansion_half(b, 0, qreps_b)
        late_exp = lambda bb=b, qr=qreps_b: expansion_half(bb, 1, qr)
        if b + 1 < NBLK:
            av_cur = av_next
    late_exp()

    nc.sync.dma_start(outp[:, :], out_sb[:, :])


def kernel(emb, W, b, unnorm_trans, state_priors):
    emb = np.asarray(emb, dtype=np.float32)
    W = np.asarray(W, dtype=np.float32)
    b = np.asarray(b, dtype=np.float32)
    unnorm_trans = np.asarray(unnorm_trans, dtype=np.float32)
    state_priors = np.asarray(state_priors, dtype=np.float32)

    E4 = ml_dtypes.float8_e4m3

    def to_e4(x):
        return np.clip(x, -240.0, 240.0).astype(E4)

    # host-side constants
    ut = unnorm_trans - unnorm_trans.max(axis=-1, keepdims=True)
    e = np.exp(ut)
    trans_half = (0.5 * e / e.sum(axis=-1, keepdims=True)).astype(np.float32)
    transq = np.tile(trans_half, (2, 1)).astype(ml_dtypes.bfloat16)
    identb = np.tile(np.eye(H, dtype=np.float32), (2, 1)).astype(ml_dtypes.bfloat16)
    identf = np.eye(H, dtype=np.float32)
    onesr = np.ones((1, 128), dtype=np.float32)
    pr = np.exp(state_priors).astype(np.float32)
    priors_col = np.tile(pr[:, None], (2, 1)).astype(np.float32)
    priors_row = pr[None, :].astype(np.float32)

    # W -> wq[p, k2, u, q*128 + h*64 + i] = W[k2*256+u*128+p, i*64 + q + 32h]
    Wr = W.reshape(D, H, 2, 32)                # [d, i, h, q]
    Wp = Wr.transpose(0, 3, 2, 1).reshape(D, H * H)  # [d, (q, h, i)]
    wq = np.ascontiguousarray(
        Wp.reshape(NK2, 2, 128, H * H).transpose(2, 0, 1, 3).reshape(128, NK2 * 2 * H * H)
    )
    wq = to_e4(wq)
    # bias bq[h*64+i, q] = b[i*64 + q + 32h]
    br = b.reshape(H, 2, 32)                   # [i, h, q]
    bq = np.ascontiguousarray(br.transpose(1, 0, 2).reshape(128, 32)).astype(np.float32)

    nc = build_bass()

    in_maps = []
    for core in range(NCORES):
        emb_c = emb[core * BLOC : (core + 1) * BLOC]       # [2, T, D]
        ef = emb_c.reshape(BLOC * T, D).T                  # [d, tg]
        embq = np.ascontiguousarray(
            ef.reshape(NK2, 2, 128, BLOC * T).transpose(2, 0, 1, 3).reshape(128, -1)
        )
        in_maps.append(
            {
                "embq": to_e4(embq),
                "wq": wq,
                "bq": bq,
                "transq": transq,
                "identb": identb,
                "identf": identf,
                "onesr": onesr,
                "priors_col": priors_col,
                "priors_row": priors_row,
            }
        )

    import os

    trace = bool(int(os.environ.get("KERNEL_TRACE", "0")))
    res = run_bass_kernel_spmd(nc, in_maps, list(range(NCORES)), trace=trace)
    global LAST_RESULTS
    LAST_RESULTS = res
    if trace and res.exec_time_ns is not None:
        print(f"HW exec time: {res.exec_time_ns} ns")
        print(f"  mean across cores: {res.mean_exec_time_ns} ns")

    outs = []
    for core in range(NCORES):
        o = res.results[core]["outp"].astype(np.float32)   # [128, 1024]
        o = o.reshape(2, 64, NBLK, NF, L)                  # [hh, j, blk, f, s]
        arr = o.transpose(2, 0, 3, 4, 1)                   # [blk, hh, f, s, j]
        arr = arr.reshape(BLOC, T, H)                      # blk=(bloc,th); t=(th,hh,f,s)
        outs.append(arr)
    h = np.log(np.maximum(np.concatenate(outs, axis=0), 1e-30)).astype(np.float32)
    h[:, 0, :] = state_priors[None, :]
    return h


# revision 13
# speedup vs baseline: 1.2441x; 1.1203x over previous
"""Neural-HMM forward kernel for Trainium2 (8 NeuronCores, SPMD data-parallel over batch).

Math: per (b, t) a 64x64 transition matrix A_t = 0.5*softmax(emb@W + b) +
0.5*softmax(unnorm_trans); the scan h_t = logsumexp_i(h_{t-1}[i] + log A_t[i,j])
is run in plain probability space (A_t is row-stochastic, mass conserved):
p_t = A_t^T p_{t-1}, h = log p (log taken on host).

Structure per core (2 batch rows, 4 blocks of 512 timesteps):
  stage 1  logits via fp8-e4m3 DoubleRow matmuls (K=1024 as 4x256), Exp on ACT
           into e[(h,i), (q, t)] with contiguous 128-channel writes, rowsum S
           accumulated incrementally on DVE, A_nn = E * (0.5/S).
  relayout SBUF->SBUF DMA permutes e[(h,i), (q,t)] -> av[(hh,i), (j, t')]
           (512B runs); + 0.5*trans added on DVE; strided LDWEIGHTS reads of
           av are free on the PE.
  stage 2  chunked prefix chains: 8 chunks x 64 steps per block,
           N_s = A^T N_{s-1} on the PE (two 64x64 quadrants via tile_position),
           PSUM->SBUF copies split across DVE (half) and ACT (half).
  stage 3  serial boundary scan over chunk products (tiny PE matmuls).
  stage 4  p = N q expansion on DVE; p (bf16) DMA'd out; host takes log.
Stage-1 matmuls of block b+1 are interleaved into the chain loop of block b
to keep the PE busy during the chain's PSUM round-trips.
"""

import numpy as np
import ml_dtypes
import sys

sys.path.insert(0, "/opt/trn_rl_repo")

import concourse.bass as bass
import concourse.bacc as bacc
import concourse.tile as tile
from concourse import mybir
from concourse.bass_utils import run_bass_kernel_spmd

F32 = mybir.dt.float32
BF16 = mybir.dt.bfloat16
FP8 = mybir.dt.float8e4

B, T, D, H = 16, 1024, 1024, 64
NCORES = 8
BLOC = B // NCORES          # 2 batch rows per core
NBLK = 4                    # blocks per core (bloc, t-half)
BT = 512                    # timesteps per block
NQ = 32                     # stage-1 m-tiles (128 HH-columns each)
NK2 = 4                     # fp8 DoubleRow contraction tiles (256 each)
L = 64                      # chain steps per chunk
NF = 4                      # chunks per partition half; total 8 chunks/block


def build_bass():
    nc = bacc.Bacc(
        "TRN2", target_bir_lowering=False, debug=False, num_devices=NCORES
    )
    embq = nc.declare_dram_parameter("embq", [128, NK2 * 2 * BLOC * T], FP8, isOutput=False)
    wq = nc.declare_dram_parameter("wq", [128, NK2 * 2 * H * H], FP8, isOutput=False)
    bq = nc.declare_dram_parameter("bq", [128, NQ], F32, isOutput=False)
    transq = nc.declare_dram_parameter("transq", [128, H], BF16, isOutput=False)
    identb = nc.declare_dram_parameter("identb", [128, H], BF16, isOutput=False)
    identf = nc.declare_dram_parameter("identf", [H, H], F32, isOutput=False)
    onesr = nc.declare_dram_parameter("onesr", [1, 128], F32, isOutput=False)
    priors_col = nc.declare_dram_parameter("priors_col", [128, 1], F32, isOutput=False)
    priors_row = nc.declare_dram_parameter("priors_row", [1, H], F32, isOutput=False)
    outp = nc.declare_dram_parameter("outp", [128, NBLK * 256], BF16, isOutput=True)

    from contextlib import ExitStack

    with tile.TileContext(nc) as tc, ExitStack() as ctx:
        kernel_body(ctx, tc, embq, wq, bq, transq, identb, identf, onesr,
                    priors_col, priors_row, outp)
    nc.finalize()
    return nc


def kernel_body(ctx, tc, embq, wq, bq, transq, identb, identf, onesr,
                priors_col, priors_row, outp):
    nc = tc.nc

    const_pool = ctx.enter_context(tc.tile_pool(name="const", bufs=1))
    big_pool = ctx.enter_context(tc.tile_pool(name="big", bufs=1))
    av_pool = ctx.enter_context(tc.tile_pool(name="av", bufs=2))
    s_pool = ctx.enter_context(tc.tile_pool(name="s", bufs=2))
    q_pool = ctx.enter_context(tc.tile_pool(name="q", bufs=4))
    qrep_pool = ctx.enter_context(tc.tile_pool(name="qrep", bufs=2))
    prod_pool = ctx.enter_context(tc.tile_pool(name="prod", bufs=1))
    mm_psum = ctx.enter_context(tc.tile_pool(name="mmps", bufs=2, space="PSUM"))
    ch_psum = ctx.enter_context(tc.tile_pool(name="chps", bufs=2, space="PSUM"))
    bnd_psum = ctx.enter_context(tc.tile_pool(name="bndps", bufs=2, space="PSUM"))

    # ---- constants
    transq_sb = const_pool.tile([128, H], BF16)
    nc.sync.dma_start(transq_sb[:, :], transq[:, :])
    ident_dma = const_pool.tile([128, H], BF16)
    nc.sync.dma_start(ident_dma[:, :], identb[:, :])
    ident_sb = const_pool.tile([128, H], BF16)
    nc.vector.tensor_copy(ident_sb[:, :], ident_dma[:, :])
    identf_dma = const_pool.tile([H, H], F32)
    nc.sync.dma_start(identf_dma[:, :], identf[:, :])
    identf_sb = const_pool.tile([H, H], F32)
    nc.vector.tensor_copy(identf_sb[:, :], identf_dma[:, :])
    ones_dma = const_pool.tile([1, 128], F32)
    nc.sync.dma_start(ones_dma[:, :], onesr[:, :])
    ones_sb = const_pool.tile([1, 128], F32)
    nc.vector.tensor_copy(ones_sb[:, :], ones_dma[:, :])
    pcol_sb = const_pool.tile([128, 1], F32)
    nc.sync.dma_start(pcol_sb[:, :], priors_col[:, :])
    prow_sb = const_pool.tile([1, H], F32)
    nc.sync.dma_start(prow_sb[:, :], priors_row[:, :])
    bq_sb = const_pool.tile([128, NQ], F32)
    nc.sync.dma_start(bq_sb[:, :], bq[:, :])

    # trans pre-expanded over a half t'-range: trx[p, j, t128] = 0.5*trans[i, j]
    trx = const_pool.tile([128, H * 64], BF16)
    nc.vector.tensor_copy(
        trx[:, :].rearrange("p (j t) -> p j t", j=H),
        transq_sb[:, :].rearrange("p (j u) -> p j u", u=1).broadcast_to([128, H, 64]),
    )

    # ---- resident inputs
    wq_sb = big_pool.tile([128, NK2 * 2 * H * H], FP8)
    nc.sync.dma_start(wq_sb[:, :], wq[:, :])
    embq_sb = big_pool.tile([128, NK2 * 2 * BLOC * T], FP8)
    nc.sync.dma_start(embq_sb[:, :], embq[:, :])
    wqv = wq_sb[:, :].rearrange("p (k u m) -> p k u m", k=NK2, u=2)
    embqv = embq_sb[:, :].rearrange("p (k u t) -> p k u t", k=NK2, u=2)

    e_sb = big_pool.tile([128, NQ * BT], BF16)      # [(h,i), (q, t)]
    ev = e_sb[:, :].rearrange("p (q t) -> p q t", q=NQ)
    nv = big_pool.tile([128, L * NF * H], BF16)     # [(hh,j), (s, f, c)]
    nvv = nv[:, :].rearrange("p (s f c) -> p s f c", s=L, f=NF)
    HEAD = 12
    nv_head = big_pool.tile([128, HEAD * NF * H], BF16)
    nhv = nv_head[:, :].rearrange("p (s f c) -> p s f c", s=HEAD, f=NF)
    out_sb = big_pool.tile([128, NBLK * 256], BF16)

    # ---- per-block state helpers
    state = {}

    def _s1_mm(ps, blk, q, k2):
        nc.tensor.matmul(
            ps[:, :],
            wqv[:, k2, :, q * 128 : (q + 1) * 128],
            embqv[:, k2, :, blk * BT : (blk + 1) * BT],
            start=(k2 == 0),
            stop=(k2 == NK2 - 1),
            perf_mode=mybir.MatmulPerfMode.DoubleRow,
            skip_group_check=True,
        )

    def stage1_units(blk):
        units = []
        for q in range(NQ):
            box = {}
            def unit_a(q=q, box=box):
                ps = mm_psum.tile([128, BT], F32, tag="mm")
                box["ps"] = ps
                _s1_mm(ps, blk, q, 0)
                _s1_mm(ps, blk, q, 1)
            def unit_b(q=q, box=box):
                ps = box["ps"]
                _s1_mm(ps, blk, q, 2)
                _s1_mm(ps, blk, q, 3)
                nc.scalar.activation(
                    ev[:, q, :], ps[:, :], mybir.ActivationFunctionType.Exp,
                    bias=bq_sb[:, q : q + 1],
                )
            units.append(unit_a)
            units.append(unit_b)
        return units

    def stage1_group(blk, q):
        ps = mm_psum.tile([128, BT], F32, tag="mm")
        for k2 in range(NK2):
            _s1_mm(ps, blk, q, k2)
        nc.scalar.activation(
            ev[:, q, :], ps[:, :], mybir.ActivationFunctionType.Exp,
            bias=bq_sb[:, q : q + 1],
        )

    def make_av(blk):
        av = av_pool.tile([128, H * 256], BF16, tag="av")
        return av

    def relayout(blk, av):
        """SBUF->SBUF DMA: e[(h,i),(q,t)] -> av[(hh,i),(j=q+32h, t')] (raw exps)."""
        dv = av[:, :].rearrange("p (j t) -> p j t", j=H)
        for hh in range(2):
            for h in range(2):
                eng = nc.sync if h == 0 else nc.gpsimd
                eng.dma_start(
                    dv[hh * 64 : hh * 64 + 64, 32 * h : 32 * h + 32, :],
                    ev[h * 64 : h * 64 + 64, :, hh * 256 : hh * 256 + 256],
                )

    def mix_on_av(blk, av):
        """rowsum over j (tree, e as scratch), r = 0.5/S, av = av*r + 0.5*trans."""
        dv = av[:, :].rearrange("p (j t) -> p j t", j=H)
        ctx2 = nc.allow_low_precision(reason="softmax denom tree in bf16")
        ctx2.__enter__()
        # 6-level tree: S[(hh,i), t'] = sum_j av[(hh,i), j, t']
        nc.vector.tensor_tensor(
            e_sb[:, 0:8192].rearrange("p (j t) -> p j t", j=32),
            dv[:, 0:32, :], dv[:, 32:64, :], op=mybir.AluOpType.add,
        )
        for w in (4096, 2048, 1024, 512):
            nc.vector.tensor_tensor(
                e_sb[:, 0:w], e_sb[:, 0:w], e_sb[:, w : 2 * w], op=mybir.AluOpType.add,
            )
        s32 = s_pool.tile([128, 256], F32, tag="s32")
        nc.vector.tensor_tensor(s32[:, :], e_sb[:, 0:256], e_sb[:, 256:512], op=mybir.AluOpType.add)
        nc.vector.reciprocal(s32[:, :], s32[:, :])
        r_sb = s_pool.tile([128, 256], BF16, tag="r")
        nc.vector.tensor_scalar_mul(r_sb[:, :], s32[:, :], 0.5)
        nc.vector.tensor_tensor(
            dv[:, :, :], dv[:, :, :],
            r_sb[:, :].rearrange("p (u t) -> p u t", u=1).broadcast_to([128, H, 256]),
            op=mybir.AluOpType.mult,
        )
        trxv = trx[:, :].rearrange("p (j t) -> p j t", j=H)
        for quarter in range(4):
            nc.vector.tensor_tensor(
                dv[:, :, quarter * 64 : quarter * 64 + 64],
                dv[:, :, quarter * 64 : quarter * 64 + 64],
                trxv[:, :, :],
                op=mybir.AluOpType.add,
            )
        ctx2.__exit__(None, None, None)

    def reset_q():
        q_col = q_pool.tile([128, 1], F32, tag="qcol")
        nc.vector.tensor_copy(q_col[:, :], pcol_sb[:, :])
        q_row = q_pool.tile([1, H], F32, tag="qrow")
        nc.vector.tensor_copy(q_row[:, :], prow_sb[:, :])
        state["q_col"] = q_col
        state["q_row"] = q_row

    def chains(blk, av, fillers, after_fillers=None, mix_fn=None, late_exp=None):
        th = blk % 2
        avv = av[:, :].rearrange("p (j t) -> p j t", j=H)
        first_global = th == 0
        has_mix = mix_fn is not None
        for s in range(L):
            cp = ch_psum.tile([128, 256], F32, tag="chp")
            cpv = cp[:, :].rearrange("p (f c) -> p f c", f=NF)
            for hh in range(2):
                for f in range(NF):
                    if s == 0:
                        if first_global and hh == 0 and f == 0:
                            continue
                        rhs = ident_sb[hh * 64 : hh * 64 + 64, :]
                    elif s <= HEAD:
                        rhs = nhv[hh * 64 : hh * 64 + 64, s - 1, f, :]
                    else:
                        rhs = nvv[hh * 64 : hh * 64 + 64, s - 1, f, :]
                    nc.tensor.matmul(
                        cp[hh * 64 : hh * 64 + 64, f * 64 : f * 64 + 64],
                        avv[hh * 64 : hh * 64 + 64, :, f * 64 + s],
                        rhs,
                        start=True,
                        stop=True,
                        tile_position=(hh * 64, hh * 64),
                    )
            # s < HEAD slots land in nv_head so the previous block's expansion
            # can still be reading nv; DVE-busy windows push copies to ACT.
            dstv = nhv if s < HEAD else nvv
            act_both = (s < HEAD) or (20 <= s < 36) or (has_mix and 40 <= s < 58)
            if s == 0 and first_global:
                nc.vector.tensor_copy(dstv[0:64, 0, 0, :], ident_sb[0:64, :])
                nc.scalar.activation(
                    dstv[0:64, 0, 1:4, :], cpv[0:64, 1:4, :],
                    mybir.ActivationFunctionType.Copy,
                )
                nc.scalar.activation(
                    dstv[64:128, 0, :, :], cpv[64:128, :, :],
                    mybir.ActivationFunctionType.Copy,
                )
            elif act_both:
                nc.scalar.activation(
                    dstv[:, s, :, :], cpv[:, :, :],
                    mybir.ActivationFunctionType.Copy,
                )
            else:
                nc.vector.tensor_copy(dstv[0:64, s, :, :], cpv[0:64, :, :])
                nc.scalar.activation(
                    dstv[64:128, s, :, :], cpv[64:128, :, :],
                    mybir.ActivationFunctionType.Copy,
                )
            if s == HEAD:
                # drain staged head slots into nv (after old expansion done)
                nc.vector.tensor_copy(nv[:, 0 : HEAD * NF * H], nv_head[:, :])
            if s == 20 and late_exp is not None:
                late_exp()
                late_exp = None
            npop = 2 if s < 24 else 1
            for _ in range(npop):
                if fillers:
                    fillers.pop(0)()
                    if not fillers and after_fillers is not None:
                        after_fillers()
                        after_fillers = None
            if not fillers and mix_fn is not None and s >= 42:
                mix_fn()
                mix_fn = None
        if after_fillers is not None:
            after_fillers()
        if mix_fn is not None:
            mix_fn()
        if late_exp is not None:
            late_exp()

    def boundary_expansion(blk, av):
        avv = av[:, :].rearrange("p (j t) -> p j t", j=H)
        qreps = []
        for f in range(NF):
            qr = qrep_pool.tile([128, H], BF16, tag=f"qr{f}")
            qreps.append(qr)
        state["qreps"] = qreps
        q_col = state["q_col"]
        q_row = state["q_row"]
        for c in range(2 * NF):
            hh, f = c // NF, c % NF
            bnd = bnd_psum.tile([128, 256], F32, tag="bnd")
            nc.tensor.matmul(bnd[:, 0:H], ones_sb[:, :], q_row[:, :], start=True, stop=True)
            nc.scalar.activation(
                qreps[f][hh * 64 : hh * 64 + 64, :], bnd[hh * 64 : hh * 64 + 64, 0:H],
                mybir.ActivationFunctionType.Copy,
            )
            # full chunk product M63 = N_62^T-chain x A_63
            nc.tensor.matmul(
                bnd[hh * 64 : hh * 64 + 64, H : 2 * H],
                nvv[hh * 64 : hh * 64 + 64, L - 2, f, :],
                avv[hh * 64 : hh * 64 + 64, :, f * 64 + (L - 1)],
                start=True,
                stop=True,
                tile_position=(hh * 64, hh * 64),
            )
            m63_sb = q_pool.tile([128, H], F32, tag="m63")
            nc.vector.tensor_copy(
                m63_sb[hh * 64 : hh * 64 + 64, :], bnd[hh * 64 : hh * 64 + 64, H : 2 * H]
            )
            nc.tensor.matmul(
                bnd[0:H, 128:129],
                m63_sb[hh * 64 : hh * 64 + 64, :],
                q_col[hh * 64 : hh * 64 + 64, :],
                start=True,
                stop=True,
                tile_position=(hh * 64, 0),
            )
            q_col = q_pool.tile([128, 1], F32, tag="qcol")
            nc.vector.tensor_copy(q_col[0:64, :], bnd[0:H, 128:129])
            nc.vector.tensor_copy(q_col[64:128, :], bnd[0:H, 128:129])
            nc.tensor.matmul(
                bnd[0:1, 192:256], q_col[0:64, :], identf_sb[:, :], start=True, stop=True
            )
            q_row = q_pool.tile([1, H], F32, tag="qrow")
            nc.vector.tensor_copy(q_row[:, :], bnd[0:1, 192:256])
        state["q_col"] = q_col
        state["q_row"] = q_row

    def expansion_half(blk, sh2, qreps):
        with nc.allow_low_precision(reason="p in bf16; log on host"):
            for f in range(NF):
                prod = prod_pool.tile([128, (L // 2) * H], BF16, tag="prod")
                pv = prod[:, :].rearrange("p (s c) -> p s c", s=L // 2)
                nc.vector.tensor_tensor(
                    pv,
                    nvv[:, sh2 * 32 : sh2 * 32 + 32, f, :],
                    qreps[f][:, :].rearrange("p (u c) -> p u c", u=1).broadcast_to([128, L // 2, H]),
                    op=mybir.AluOpType.mult,
                )
                o0 = blk * 256 + f * 64 + sh2 * 32
                nc.vector.tensor_reduce(
                    out=out_sb[:, o0 : o0 + 32],
                    in_=pv,
                    axis=mybir.AxisListType.X,
                    op=mybir.AluOpType.add,
                )

    # ---- pipelined schedule
    # prologue: stage 1 of block 0
    av_cur = make_av(0)
    for q in range(NQ):
        stage1_group(0, q)
    relayout(0, av_cur)
    mix_on_av(0, av_cur)
    reset_q()

    late_exp = None
    for b in range(NBLK):
        if b == 2:
            reset_q()
        if b + 1 < NBLK:
            av_next = make_av(b + 1)
            fillers = stage1_units(b + 1)
            after = lambda nb=b + 1, avn=av_next: relayout(nb, avn)
            mix_fn = lambda nb=b + 1, avn=av_next: mix_on_av(nb, avn)
        else:
            av_next = None
            fillers = []
            after = None
            mix_fn = None
        chains(b, av_cur, fillers, after, mix_fn, late_exp)
        boundary_expansion(b, av_cur)
        qreps_b = state["qreps"]
        expansion_half(b, 0, qreps_b)
        late_exp = lambda bb=b, qr=qreps_b: expansion_half(bb, 1, qr)
        if b + 1 < NBLK:
            av_cur = av_next
    late_exp()

    nc.sync.dma_start(outp[:, :], out_sb[:, :])


def kernel(emb, W, b, unnorm_trans, state_priors):
    emb = np.asarray(emb, dtype=np.float32)
    W = np.asarray(W, dtype=np.float32)
    b = np.asarray(b, dtype=np.float32)
    unnorm_trans = np.asarray(unnorm_trans, dtype=np.float32)
    state_priors = np.asarray(state_priors, dtype=np.float32)

    E4 = ml_dtypes.float8_e4m3

    def to_e4(x):
        return np.clip(x, -240.0, 240.0).astype(E4)

    # host-side constants
    ut = unnorm_trans - unnorm_trans.max(axis=-1, keepdims=True)
    e = np.exp(ut)
    trans_half = (0.5 * e / e.sum(axis=-1, keepdims=True)).astype(np.float32)
    transq = np.tile(trans_half, (2, 1)).astype(ml_dtypes.bfloat16)
    identb = np.tile(np.eye(H, dtype=np.float32), (2, 1)).astype(ml_dtypes.bfloat16)
    identf = np.eye(H, dtype=np.float32)
    onesr = np.ones((1, 128), dtype=np.float32)
    pr = np.exp(state_priors).astype(np.float32)
    priors_col = np.tile(pr[:, None], (2, 1)).astype(np.float32)
    priors_row = pr[None, :].astype(np.float32)

    # W -> wq[p, k2, u, q*128 + h*64 + i] = W[k2*256+u*128+p, i*64 + q + 32h]
    Wr = W.reshape(D, H, 2, 32)                # [d, i, h, q]
    Wp = Wr.transpose(0, 3, 2, 1).reshape(D, H * H)  # [d, (q, h, i)]
    wq = np.ascontiguousarray(
        Wp.reshape(NK2, 2, 128, H * H).transpose(2, 0, 1, 3).reshape(128, NK2 * 2 * H * H)
    )
    wq = to_e4(wq)
    # bias bq[h*64+i, q] = b[i*64 + q + 32h]
    br = b.reshape(H, 2, 32)                   # [i, h, q]
    bq = np.ascontiguousarray(br.transpose(1, 0, 2).reshape(128, 32)).astype(np.float32)

    nc = build_bass()

    in_maps = []
    for core in range(NCORES):
        emb_c = emb[core * BLOC : (core + 1) * BLOC]       # [2, T, D]
        ef = emb_c.reshape(BLOC * T, D).T                  # [d, tg]
        embq = np.ascontiguousarray(
            ef.reshape(NK2, 2, 128, BLOC * T).transpose(2, 0, 1, 3).reshape(128, -1)
        )
        in_maps.append(
            {
                "embq": to_e4(embq),
                "wq": wq,
                "bq": bq,
                "transq": transq,
                "identb": identb,
                "identf": identf,
                "onesr": onesr,
                "priors_col": priors_col,
                "priors_row": priors_row,
            }
        )

    import os

    trace = bool(int(os.environ.get("KERNEL_TRACE", "0")))
    res = run_bass_kernel_spmd(nc, in_maps, list(range(NCORES)), trace=trace)
    global LAST_RESULTS
    LAST_RESULTS = res
    if trace and res.exec_time_ns is not None:
        print(f"HW exec time: {res.exec_time_ns} ns")
        print(f"  mean across cores: {res.mean_exec_time_ns} ns")

    outs = []
    for core in range(NCORES):
        o = res.results[core]["outp"].astype(np.float32)   # [128, 1024]
        o = o.reshape(2, 64, NBLK, NF, L)                  # [hh, j, blk, f, s]
        arr = o.transpose(2, 0, 3, 4, 1)                   # [blk, hh, f, s, j]
        arr = arr.reshape(BLOC, T, H)                      # blk=(bloc,th); t=(th,hh,f,s)
        outs.append(arr)
    h = np.log(np.maximum(np.concatenate(outs, axis=0), 1e-30)).astype(np.float32)
    h[:, 0, :] = state_priors[None, :]
    return h
